# revision 5
# baseline (speedup 1.0000x reference)
"""GNN message-passing (gather + segment-sum) on 8 TRN2 NeuronCores.

Strategy (edge-parallel, destination-sharded):
  - Host: shard edges by destination node range (6250 nodes per core), bucket
    each core's edges by 128-node destination block, split each bucket by
    source half (src < 25000 vs >= 25000, for int16 gather indices), pad
    buckets to a common per-block tile count (multiple of 128 edges).
  - Device (per core, SPMD):
      for each 128-node block:
        dma_gather   msgs[128e, T, 64] = x[src]          (2 calls: lo/hi half)
        DVE is_equal sel[128e, T, 128] = (dst_rel == iota)
        PE matmul    psum[64, 128] += msgs[:,t,:].T @ sel[:,t,:]   (T accums)
        DVE copy     outT[64, block*128:...] = psum
      one DMA of outT [64, 6272] to DRAM.
  - Host: out[c*6250:(c+1)*6250] = outT_c.T[:6250]  (concatenate cores).

Collision-free by construction (no scatter): duplicate destinations are
combined by the one-hot matmul accumulation in PSUM.
"""

import numpy as np

import concourse.tile as tile
from concourse import bacc, mybir
from concourse import bass_utils

N_NODES = 50000
D = 64
N_CORES = 8
NODES_PER_CORE = N_NODES // N_CORES  # 6250
BLOCK = 128
N_BLOCKS = -(-NODES_PER_CORE // BLOCK)  # 49
OUT_COLS = N_BLOCKS * BLOCK  # 6272
SRC_SPLIT = 25000
PAD_REL = 999.0  # dst_rel value for padding edges (matches no iota column)


def bin_edges(edge_index, n_cores, nodes_per_core, block, src_split):
    """Bin edges into (core, dst-block, src-half) buckets, pad to common
    per-(block,half) tile counts.

    Returns:
      T_lo, T_hi: [n_blocks] int arrays, tiles (128 edges) per bucket
      src16: [n_cores, 128, tot_cols] int16 gather indices (wrapped+replicated)
      dstrel: [n_cores, 128, tot_tiles] float32 relative dst (position-major)
    """
    dst = np.asarray(edge_index[0], dtype=np.int64)
    src = np.asarray(edge_index[1], dtype=np.int64)
    n_blocks = -(-nodes_per_core // block)

    core = dst // nodes_per_core
    local = dst - core * nodes_per_core
    blk = local // block
    rel = (local - blk * block).astype(np.float32)
    half = (src >= src_split).astype(np.int64)
    bucket = (core * n_blocks + blk) * 2 + half

    order = np.argsort(bucket, kind="stable")
    src_s = src[order]
    rel_s = rel[order]
    bucket_s = bucket[order]

    counts = np.bincount(bucket, minlength=n_cores * n_blocks * 2).reshape(
        n_cores, n_blocks, 2
    )
    # common (across cores) tile count per (block, half); at least 1
    T = np.maximum(1, -(-counts.max(axis=0) // 128))  # [n_blocks, 2]
    T_lo, T_hi = T[:, 0], T[:, 1]
    tiles_per_block = T_lo + T_hi
    tot_tiles = int(tiles_per_block.sum())
    tot_edges = tot_tiles * 128

    # bucket start offsets (in padded edge positions), same for every core
    pad_sizes = T.reshape(-1) * 128  # [n_blocks*2] in block-major, half-minor
    pad_starts = np.zeros(n_blocks * 2, dtype=np.int64)
    pad_starts[1:] = np.cumsum(pad_sizes)[:-1]

    src_pad = np.zeros((n_cores, tot_edges), dtype=np.int16)
    rel_pad = np.full((n_cores, tot_edges), PAD_REL, dtype=np.float32)

    cum = counts.reshape(n_cores, -1).cumsum(axis=1)
    starts_real = np.zeros((n_cores, n_blocks * 2), dtype=np.int64)
    starts_real[:, 1:] = cum[:, :-1]
    # per-core offset of this core's first edge in the sorted arrays
    core_base = np.zeros(n_cores, dtype=np.int64)
    core_counts = counts.sum(axis=(1, 2))
    core_base[1:] = np.cumsum(core_counts)[:-1]

    for c in range(n_cores):
        cnts = counts[c].reshape(-1)
        for bh in range(n_blocks * 2):
            n = int(cnts[bh])
            if n == 0:
                continue
            s = int(core_base[c] + starts_real[c, bh])
            p = int(pad_starts[bh])
            sv = src_s[s : s + n]
            if bh % 2 == 1:
                sv = sv - src_split
            src_pad[c, p : p + n] = sv.astype(np.int16)
            rel_pad[c, p : p + n] = rel_s[s : s + n]

    # gather indices: wrapped in 16 partitions (idx i -> [i%16, i//16]),
    # replicated to all 8 gpsimd partition groups
    w = src_pad.reshape(n_cores, -1, 16).transpose(0, 2, 1)  # [n_cores,16,cols]
    src16 = np.tile(w, (1, 8, 1)).copy()  # [n_cores, 128, cols]

    # dst_rel: edge position-major: position i -> [i%128, i//128]
    dstrel = (
        rel_pad.reshape(n_cores, -1, 128).transpose(0, 2, 1).copy()
    )  # [n_cores, 128, tot_tiles]

    return T_lo.astype(int), T_hi.astype(int), src16, dstrel


def build_program(T_lo, T_hi, n_rows, src_split, d=D, block=BLOCK, repeat=1):
    """Build the SPMD Bass program for given per-block tile counts.

    repeat > 1 wraps the whole block loop in a hardware For_i loop running
    the identical computation `repeat` times (for device-time measurement
    by wall-clock slope; the result is unchanged since every iteration
    overwrites the same outputs).
    """
    n_blocks = len(T_lo)
    out_cols = n_blocks * block
    tot_tiles = int((T_lo + T_hi).sum())

    nc = bacc.Bacc(
        "TRN2",
        target_bir_lowering=False,
        debug=False,
        num_devices=N_CORES,
        num_swdge_queues=4,
    )
    x = nc.dram_tensor("x", [n_rows, d], mybir.dt.float32, kind="ExternalInput")
    src16 = nc.dram_tensor(
        "src16", [128, tot_tiles * 8], mybir.dt.int16, kind="ExternalInput"
    )
    dstrel = nc.dram_tensor(
        "dstrel", [128, tot_tiles], mybir.dt.float32, kind="ExternalInput"
    )
    iota_in = nc.dram_tensor("iota", [128, block], mybir.dt.float32, kind="ExternalInput")
    out = nc.dram_tensor("out", [d, out_cols], mybir.dt.float32, kind="ExternalOutput")

    x_lo = x.ap()[0:src_split, :]
    x_hi = x.ap()[src_split:n_rows, :]

    with tile.TileContext(nc) as tc:
        with (
            tc.tile_pool(name="meta", bufs=1) as meta_pool,
            tc.tile_pool(name="msgs", bufs=3) as msgs_pool,
            tc.tile_pool(name="sel", bufs=3) as sel_pool,
            tc.tile_pool(name="obuf", bufs=1) as obuf_pool,
            tc.tile_pool(name="psum", bufs=4, space="PSUM") as psum_pool,
        ):
            src_t = meta_pool.tile([128, tot_tiles * 8], mybir.dt.int16)
            nc.sync.dma_start(src_t[:], src16.ap())
            rel_t = meta_pool.tile([128, tot_tiles], mybir.dt.float32)
            nc.sync.dma_start(rel_t[:], dstrel.ap())
            iota_t = meta_pool.tile([128, block], mybir.dt.float32)
            nc.sync.dma_start(iota_t[:], iota_in.ap())

            outbuf = obuf_pool.tile([d, out_cols], mybir.dt.float32)

            T_max = int(max((T_lo + T_hi).max(), 1))

            def body():
                off = 0  # tile offset of current bucket
                q = 0
                for b in range(n_blocks):
                    tl, th = int(T_lo[b]), int(T_hi[b])
                    tb = tl + th
                    msgs = msgs_pool.tile(
                        [128, T_max, d], mybir.dt.float32, tag="msgs"
                    )
                    nc.gpsimd.dma_gather(
                        msgs[:, 0:tl, :],
                        x_lo,
                        src_t[:, off * 8 : (off + tl) * 8],
                        tl * 128,
                        tl * 128,
                        d,
                        queue_num=q % 4,
                        single_packet=False,
                    )
                    q += 1
                    nc.gpsimd.dma_gather(
                        msgs[:, tl:tb, :],
                        x_hi,
                        src_t[:, (off + tl) * 8 : (off + tb) * 8],
                        th * 128,
                        th * 128,
                        d,
                        queue_num=q % 4,
                        single_packet=False,
                    )
                    q += 1

                    sel = sel_pool.tile(
                        [128, T_max, block], mybir.dt.float32, tag="sel"
                    )
                    nc.vector.tensor_tensor(
                        out=sel[:, 0:tb, :],
                        in0=rel_t[:, off : off + tb].to_broadcast([128, tb, block]),
                        in1=iota_t[:]
                        .rearrange("p (o n) -> p o n", o=1)
                        .to_broadcast([128, tb, block]),
                        op=mybir.AluOpType.is_equal,
                    )

                    psum = psum_pool.tile([d, block], mybir.dt.float32, space="PSUM")
                    for t in range(tb):
                        nc.tensor.matmul(
                            out=psum[:],
                            lhsT=msgs[:, t, :],
                            rhs=sel[:, t, :],
                            start=(t == 0),
                            stop=(t == tb - 1),
                        )
                    nc.vector.tensor_copy(
                        out=outbuf[:, b * block : (b + 1) * block], in_=psum[:]
                    )
                    off += tb
                nc.sync.dma_start(out.ap(), outbuf[:])

            if repeat > 1:
                with tc.For_i(0, repeat, 1):
                    body()
            else:
                body()

    nc.compile()
    return nc


def kernel(edge_index, x):
    edge_index = np.asarray(edge_index)
    x = np.ascontiguousarray(np.asarray(x, dtype=np.float32))
    T_lo, T_hi, src16, dstrel = bin_edges(
        edge_index, N_CORES, NODES_PER_CORE, BLOCK, SRC_SPLIT
    )
    nc = build_program(T_lo, T_hi, N_NODES, SRC_SPLIT)

    iota = np.broadcast_to(
        np.arange(BLOCK, dtype=np.float32)[None, :], (128, BLOCK)
    ).copy()
    in_maps = [
        {"x": x, "src16": src16[c], "dstrel": dstrel[c], "iota": iota}
        for c in range(N_CORES)
    ]
    res = bass_utils.run_bass_kernel_spmd(nc, in_maps, core_ids=list(range(N_CORES)))

    out = np.empty((N_NODES, D), dtype=np.float32)
    for c in range(N_CORES):
        out[c * NODES_PER_CORE : (c + 1) * NODES_PER_CORE] = res.results[c]["out"].T[
            :NODES_PER_CORE
        ]
    return out


# revision 6
# speedup vs baseline: 1.4578x; 1.4578x over previous
"""GNN message-passing (gather + segment-sum) on 8 TRN2 NeuronCores.

Strategy (edge-parallel, destination-sharded):
  - Host: shard edges by destination node range (6250 nodes per core), bucket
    each core's edges by 128-node destination block, split each bucket by
    source half (src < 25000 vs >= 25000, for int16 gather indices), pad
    buckets to a common per-block tile count (multiple of 128 edges).
  - Device (per core, SPMD):
      for each 128-node block:
        dma_gather   msgs[128e, T, 64] = x[src]          (2 calls: lo/hi half)
        DVE is_equal sel[128e, T, 128] = (dst_rel == iota)
        PE matmul    psum[64, 128] += msgs[:,t,:].T @ sel[:,t,:]   (T accums)
        DVE copy     outT[64, block*128:...] = psum
      one DMA of outT [64, 6272] to DRAM.
  - Host: out[c*6250:(c+1)*6250] = outT_c.T[:6250]  (concatenate cores).

Collision-free by construction (no scatter): duplicate destinations are
combined by the one-hot matmul accumulation in PSUM.
"""

import numpy as np

import concourse.tile as tile
from concourse import bacc, mybir
from concourse import bass_utils

N_NODES = 50000
D = 64
N_CORES = 8
NODES_PER_CORE = N_NODES // N_CORES  # 6250
BLOCK = 128
N_BLOCKS = -(-NODES_PER_CORE // BLOCK)  # 49
OUT_COLS = N_BLOCKS * BLOCK  # 6272
SRC_SPLIT = 25000
PAD_REL = 999.0  # dst_rel value for padding edges (matches no iota column)


def bin_edges(edge_index, n_cores, nodes_per_core, block, src_split):
    """Bin edges into (core, dst-block, src-half) buckets, pad to common
    per-(block,half) tile counts.

    Returns:
      T_lo, T_hi: [n_blocks] int arrays, tiles (128 edges) per bucket
      src16: [n_cores, 128, tot_cols] int16 gather indices (wrapped+replicated)
      dstrel: [n_cores, 128, tot_tiles] float32 relative dst (position-major)
    """
    dst = np.asarray(edge_index[0], dtype=np.int64)
    src = np.asarray(edge_index[1], dtype=np.int64)
    n_blocks = -(-nodes_per_core // block)

    core = dst // nodes_per_core
    local = dst - core * nodes_per_core
    blk = local // block
    rel = (local - blk * block).astype(np.float32)
    half = (src >= src_split).astype(np.int64)
    bucket = (core * n_blocks + blk) * 2 + half

    order = np.argsort(bucket, kind="stable")
    src_s = src[order]
    rel_s = rel[order]
    bucket_s = bucket[order]

    counts = np.bincount(bucket, minlength=n_cores * n_blocks * 2).reshape(
        n_cores, n_blocks, 2
    )
    # common (across cores) tile count per (block, half); at least 1
    T = np.maximum(1, -(-counts.max(axis=0) // 128))  # [n_blocks, 2]
    T_lo, T_hi = T[:, 0], T[:, 1]
    tiles_per_block = T_lo + T_hi
    tot_tiles = int(tiles_per_block.sum())
    tot_edges = tot_tiles * 128

    # bucket start offsets (in padded edge positions), same for every core
    pad_sizes = T.reshape(-1) * 128  # [n_blocks*2] in block-major, half-minor
    pad_starts = np.zeros(n_blocks * 2, dtype=np.int64)
    pad_starts[1:] = np.cumsum(pad_sizes)[:-1]

    src_pad = np.zeros((n_cores, tot_edges), dtype=np.int16)
    rel_pad = np.full((n_cores, tot_edges), PAD_REL, dtype=np.float32)

    cum = counts.reshape(n_cores, -1).cumsum(axis=1)
    starts_real = np.zeros((n_cores, n_blocks * 2), dtype=np.int64)
    starts_real[:, 1:] = cum[:, :-1]
    # per-core offset of this core's first edge in the sorted arrays
    core_base = np.zeros(n_cores, dtype=np.int64)
    core_counts = counts.sum(axis=(1, 2))
    core_base[1:] = np.cumsum(core_counts)[:-1]

    for c in range(n_cores):
        cnts = counts[c].reshape(-1)
        for bh in range(n_blocks * 2):
            n = int(cnts[bh])
            if n == 0:
                continue
            s = int(core_base[c] + starts_real[c, bh])
            p = int(pad_starts[bh])
            sv = src_s[s : s + n]
            if bh % 2 == 1:
                sv = sv - src_split
            src_pad[c, p : p + n] = sv.astype(np.int16)
            rel_pad[c, p : p + n] = rel_s[s : s + n]

    # gather indices: wrapped in 16 partitions (idx i -> [i%16, i//16]),
    # replicated to all 8 gpsimd partition groups
    w = src_pad.reshape(n_cores, -1, 16).transpose(0, 2, 1)  # [n_cores,16,cols]
    src16 = np.tile(w, (1, 8, 1)).copy()  # [n_cores, 128, cols]

    # dst_rel: edge position-major: position i -> [i%128, i//128]
    dstrel = (
        rel_pad.reshape(n_cores, -1, 128).transpose(0, 2, 1).copy()
    )  # [n_cores, 128, tot_tiles]

    return T_lo.astype(int), T_hi.astype(int), src16, dstrel


def build_program(T_lo, T_hi, n_rows, src_split, d=D, block=BLOCK, repeat=1):
    """Build the SPMD Bass program for given per-block tile counts.

    repeat > 1 wraps the whole block loop in a hardware For_i loop running
    the identical computation `repeat` times (for device-time measurement
    by wall-clock slope; the result is unchanged since every iteration
    overwrites the same outputs).
    """
    n_blocks = len(T_lo)
    out_cols = n_blocks * block
    tot_tiles = int((T_lo + T_hi).sum())

    nc = bacc.Bacc(
        "TRN2",
        target_bir_lowering=False,
        debug=False,
        num_devices=N_CORES,
        num_swdge_queues=4,
    )
    x = nc.dram_tensor("x", [n_rows, d], mybir.dt.float32, kind="ExternalInput")
    src16 = nc.dram_tensor(
        "src16", [128, tot_tiles * 8], mybir.dt.int16, kind="ExternalInput"
    )
    dstrel = nc.dram_tensor(
        "dstrel", [128, tot_tiles], mybir.dt.float32, kind="ExternalInput"
    )
    iota_in = nc.dram_tensor("iota", [128, block], mybir.dt.float32, kind="ExternalInput")
    out = nc.dram_tensor("out", [d, out_cols], mybir.dt.float32, kind="ExternalOutput")

    x_lo = x.ap()[0:src_split, :]
    x_hi = x.ap()[src_split:n_rows, :]

    with tile.TileContext(nc) as tc:
        with (
            tc.tile_pool(name="meta", bufs=1) as meta_pool,
            tc.tile_pool(name="msgs", bufs=6) as msgs_pool,
            tc.tile_pool(name="sel", bufs=4) as sel_pool,
            tc.tile_pool(name="obuf", bufs=1) as obuf_pool,
            tc.tile_pool(name="psum", bufs=8, space="PSUM") as psum_pool,
        ):
            src_t = meta_pool.tile([128, tot_tiles * 8], mybir.dt.int16)
            nc.sync.dma_start(src_t[:], src16.ap())
            rel_t = meta_pool.tile([128, tot_tiles], mybir.dt.float32)
            nc.sync.dma_start(rel_t[:], dstrel.ap())
            iota_t = meta_pool.tile([128, block], mybir.dt.float32)
            nc.sync.dma_start(iota_t[:], iota_in.ap())

            outbuf = obuf_pool.tile([d, out_cols], mybir.dt.float32)

            T_max = int(max((T_lo + T_hi).max(), 1))

            def body():
                off = 0  # tile offset of current bucket
                q = 0
                for b in range(n_blocks):
                    tl, th = int(T_lo[b]), int(T_hi[b])
                    tb = tl + th
                    msgs = msgs_pool.tile(
                        [128, T_max, d], mybir.dt.float32, tag="msgs"
                    )
                    nc.gpsimd.dma_gather(
                        msgs[:, 0:tl, :],
                        x_lo,
                        src_t[:, off * 8 : (off + tl) * 8],
                        tl * 128,
                        tl * 128,
                        d,
                        queue_num=q % 4,
                        single_packet=False,
                    )
                    q += 1
                    nc.gpsimd.dma_gather(
                        msgs[:, tl:tb, :],
                        x_hi,
                        src_t[:, (off + tl) * 8 : (off + tb) * 8],
                        th * 128,
                        th * 128,
                        d,
                        queue_num=q % 4,
                        single_packet=False,
                    )
                    q += 1

                    sel = sel_pool.tile(
                        [128, T_max, block], mybir.dt.float32, tag="sel"
                    )
                    nc.vector.tensor_tensor(
                        out=sel[:, 0:tb, :],
                        in0=rel_t[:, off : off + tb].to_broadcast([128, tb, block]),
                        in1=iota_t[:]
                        .rearrange("p (o n) -> p o n", o=1)
                        .to_broadcast([128, tb, block]),
                        op=mybir.AluOpType.is_equal,
                    )

                    psum = psum_pool.tile([d, block], mybir.dt.float32, space="PSUM")
                    for t in range(tb):
                        nc.tensor.matmul(
                            out=psum[:],
                            lhsT=msgs[:, t, :],
                            rhs=sel[:, t, :],
                            start=(t == 0),
                            stop=(t == tb - 1),
                        )
                    nc.vector.tensor_copy(
                        out=outbuf[:, b * block : (b + 1) * block], in_=psum[:]
                    )
                    off += tb
                nc.sync.dma_start(out.ap(), outbuf[:])

            if repeat > 1:
                with tc.For_i(0, repeat, 1):
                    body()
            else:
                body()

    nc.compile()
    return nc


def kernel(edge_index, x):
    edge_index = np.asarray(edge_index)
    x = np.ascontiguousarray(np.asarray(x, dtype=np.float32))
    T_lo, T_hi, src16, dstrel = bin_edges(
        edge_index, N_CORES, NODES_PER_CORE, BLOCK, SRC_SPLIT
    )
    nc = build_program(T_lo, T_hi, N_NODES, SRC_SPLIT)

    iota = np.broadcast_to(
        np.arange(BLOCK, dtype=np.float32)[None, :], (128, BLOCK)
    ).copy()
    in_maps = [
        {"x": x, "src16": src16[c], "dstrel": dstrel[c], "iota": iota}
        for c in range(N_CORES)
    ]
    res = bass_utils.run_bass_kernel_spmd(nc, in_maps, core_ids=list(range(N_CORES)))

    out = np.empty((N_NODES, D), dtype=np.float32)
    for c in range(N_CORES):
        out[c * NODES_PER_CORE : (c + 1) * NODES_PER_CORE] = res.results[c]["out"].T[
            :NODES_PER_CORE
        ]
    return out
